# revision 8
# baseline (speedup 1.0000x reference)
"""Trainium2 Bass kernel for nn_LinearCondensed.

Computes out[b, o] = sum_k weight[o, k] * x[b, indx_seqs[o, k]] + bias[o]
with B=2048, IN_F=OUT_F=4096, FAN_IN=32.

Strategy: densify the sparse weight matrix on the host --
W'[o, i] = sum_{k: indx_seqs[o,k]==i} weight[o, k] -- and run a dense bf16
matmul out = x @ W'^T + bias on the PE array. OUT_F is sharded 8 ways
across cores (512 columns each), x replicated. The kernel is PE-bound:
512 N=512-equivalent matmuls ~= 110.7us of back-to-back PE streaming at
the 512cyc/2.4GHz+NX roofline (measured 216ns/MM steady, LDWEIGHTS
hidden by the PE background weight buffer).

v3 (trace-driven): the head is DMA-gated, so x is host-tiled K-MAJOR in
two 8-b-tile interleaved halves. Phase 1 runs k-outer across 8 b-tiles:
each k-step needs one 256KB x k-slab + one 131KB weight chunk, so real
matmuls start as soon as ~390KB lands (~9us) instead of waiting 2.1MB
for the old 2-wide k-outer phase (~15.4us). Per-k-step feed is 224GB/s
vs ~280-358GB/s single-queue DMA, leaving slack everywhere after the
first step. Phase 2 = k-outer over b-tiles 8-14 (x half B streams in
far ahead); phase 3 = b-tile 15 k-inner in [384|128]-wide PSUM groups,
drained onto two different DMA queues so the post-matmul tail is just
two small stores + receipt. All 8 PSUM banks carry one "acc"-tagged
rotation (warm-up dummies share the rotation, so no dedicated bank).
Bias is folded into the PSUM drain. fp8 DoubleRow would be ~1.5x PE but
fails the 2e-2 gate (3-5e-2); bf16 measures rel_err 3.0e-3.
"""

import os
import sys
import types

import ml_dtypes
import numpy as np

import concourse.bacc as bacc
import concourse.mybir as mybir
import concourse.tile as tile
from concourse.bass_utils import run_bass_kernel_spmd

B, IN_F, OUT_F, FAN_IN = 2048, 4096, 4096, 32
NCORES = 8
OSH = OUT_F // NCORES          # 512 output features per core
P = 128                        # partitions
BT = B // P                    # 16 batch tiles
KT = IN_F // P                 # 32 contraction tiles
N = OSH                        # 512 moving columns (max for fp32 PSUM bank)
WG = 4                         # k-tiles per weight DMA group
NG = KT // WG                  # 8 weight groups
HB = BT // 2                   # 8 b-tiles per k-major half
NDUMMY = 15                    # N=256 warm-up matmuls bridge boot -> data (~3.2us)

f32 = mybir.dt.float32
bf16 = mybir.dt.bfloat16

_cache = {}


def _enable_ntff_hook():
    """Register the ctypes NTFF profile hook (the image's antenv lacks
    axon_hooks); lets trace=True produce a neuron-profile under axon."""
    try:
        from antenv.axon_hooks import get_axon_ntff_profile_hook  # noqa: F401
        return
    except ImportError:
        pass
    try:
        import antenv
        from trn_agent_boot.trn_boot import _ntff_profile_via_ctypes

        mod = types.ModuleType("antenv.axon_hooks")
        holder = [None]
        mod.set_axon_ntff_profile_hook = lambda h: holder.__setitem__(0, h)
        mod.get_axon_ntff_profile_hook = lambda: holder[0]
        antenv.axon_hooks = mod
        sys.modules["antenv.axon_hooks"] = mod
        mod.set_axon_ntff_profile_hook(
            _ntff_profile_via_ctypes("/opt/axon/libaxon_pjrt.so"))
        import concourse.bass_utils as bu
        bu.upload_artifacts = lambda tmpdir: str(tmpdir)
    except Exception:
        pass


def _build():
    nc = bacc.Bacc()
    # Layouts (host-pretiled, all contiguous per partition):
    #   XK[h, p, ((a*8 + t)*128 + c)] = x[(h*8 + t)*128 + c, a*128 + p]
    #     -> k-slab (h, a): [128, 8*128] = 256KB, one DMA
    #   WT[g, p, j*512 + n] = W'[o0 + n, (4g+j)*128 + p] -> group: [128, 2048]
    XK = nc.declare_dram_parameter("XK", [2, P, KT * HB * P], bf16, isOutput=False)
    WT = nc.declare_dram_parameter("WT", [NG, P, WG * N], bf16, isOutput=False)
    BIAS = nc.declare_dram_parameter("BIAS", [P, N], f32, isOutput=False)
    OUT = nc.declare_dram_parameter("OUT", [B, N], f32, isOutput=True)

    XKv = XK.ap().rearrange("h p (a r) -> h p a r", a=KT)   # r = t*128 + c

    with tile.TileContext(nc) as tc:
        with (
            tc.tile_pool(name="xpool", bufs=1) as xpool,
            tc.tile_pool(name="wpool", bufs=1) as wpool,
            tc.tile_pool(name="cpool", bufs=1) as cpool,
            tc.tile_pool(name="opool", bufs=8) as opool,
            tc.tile_pool(name="psum", bufs=8, space="PSUM") as psum,
        ):
            # PE p-state warmup while the first k-slab + weight chunk land.
            # Dummies write into the same "acc" PSUM rotation (no extra bank).
            dl = cpool.tile([P, P], bf16)
            dr = cpool.tile([P, 256], bf16)
            nc.vector.memset(dl[:], 0)
            nc.vector.memset(dr[:], 0)
            dacc = psum.tile([P, 256], f32, name="dacc", tag="acc")
            for _ in range(NDUMMY):
                nc.tensor.matmul(dacc[:], dl[:], dr[:], start=True, stop=True)

            xs = [xpool.tile([P, KT, HB * P], bf16, tag=f"xh{h}", name=f"xh{h}")
                  for h in range(2)]
            wgroups = [wpool.tile([P, WG * N], bf16, tag=f"w{g}", name=f"w{g}")
                       for g in range(NG)]
            brow = cpool.tile([P, N], f32)

            # ---- Input DMA program: one sync (HWDGE) queue, strict order.
            # Weight chunk covering k-tile a is always dispatched before
            # x k-slab a; consumption step a needs exactly (slab a, w[a]).
            def slab(h, a, eng=None):
                (eng or (nc.sync if a % 2 == 0 else nc.scalar)).dma_start(
                    xs[h][:, a, :], XKv[h][:, a, :])

            H4 = HB * P // 2
            nc.sync.dma_start(wgroups[0][:, 0:N], WT.ap()[0][:, 0:N])
            nc.scalar.dma_start(xs[0][:, 0, 0:H4], XKv[0][:, 0, 0:H4])
            nc.scalar.dma_start(xs[0][:, 0, H4:], XKv[0][:, 0, H4:])
            nc.sync.dma_start(wgroups[0][:, N:2 * N], WT.ap()[0][:, N:2 * N])
            nc.scalar.dma_start(xs[0][:, 1, 0:H4], XKv[0][:, 1, 0:H4])
            nc.scalar.dma_start(xs[0][:, 1, H4:], XKv[0][:, 1, H4:])
            nc.sync.dma_start(wgroups[0][:, 2 * N:], WT.ap()[0][:, 2 * N:])
            slab(0, 2, nc.sync)
            slab(0, 3, nc.scalar)
            for g in range(1, NG):
                nc.sync.dma_start(wgroups[g][:], WT.ap()[g])
                for a in range(WG * g, WG * (g + 1)):
                    slab(0, a)
            nc.sync.dma_start(brow[:], BIAS.ap())
            for a in range(KT):
                slab(1, a)

            wtiles = [wgroups[a // WG][:, (a % WG) * N:(a % WG + 1) * N]
                      for a in range(KT)]

            # bias folded into the PSUM drain (bias row pre-replicated
            # across partitions on host)
            def finish_tile(t, acc, queue=None):
                osb = opool.tile([P, N], f32, tag="osb", name="osb")
                nc.vector.tensor_tensor(osb[:], acc[:], brow[:], mybir.AluOpType.add)
                (queue or nc.scalar).dma_start(OUT.ap()[t * P:(t + 1) * P, :], osb[:])

            # Phase 1: k-outer across b-tiles 0-7 (8 PSUM banks), consuming
            # each (slab, w-chunk) pair as it lands. 224GB/s steady demand.
            accs1 = [psum.tile([P, N], f32, name=f"acc{t}", tag="acc")
                     for t in range(HB)]
            for a in range(KT):
                for t in range(HB):
                    nc.tensor.matmul(
                        accs1[t][:], xs[0][:, a, t * P:(t + 1) * P], wtiles[a][:],
                        start=(a == 0), stop=(a == KT - 1),
                    )
            for t in range(HB):
                finish_tile(t, accs1[t])

            # Phase 2: k-outer across b-tiles 8-14; half-B slabs landed long
            # ago (all input DMA completes ~65us, phase 2 ends ~113us).
            accs2 = [psum.tile([P, N], f32, name=f"acc{HB + t}", tag="acc")
                     for t in range(HB - 1)]
            for a in range(KT):
                for t in range(HB - 1):
                    nc.tensor.matmul(
                        accs2[t][:], xs[1][:, a, t * P:(t + 1) * P], wtiles[a][:],
                        start=(a == 0), stop=(a == KT - 1),
                    )
            for t in range(HB - 1):
                finish_tile(HB + t, accs2[t])

            # Phase 3: last b-tile k-inner in [384|128]-wide groups; drains
            # go to two different DMA queues so the tail after the very
            # last matmul is two small parallel stores.
            t = BT - 1
            H0 = 384
            acc_a = psum.tile([P, H0], f32, name="acca", tag="acc")
            acc_b = psum.tile([P, N - H0], f32, name="accb", tag="acc")
            for a in range(KT):
                nc.tensor.matmul(
                    acc_a[:], xs[1][:, a, (HB - 1) * P:HB * P], wtiles[a][:, 0:H0],
                    start=(a == 0), stop=(a == KT - 1),
                )
                nc.tensor.matmul(
                    acc_b[:], xs[1][:, a, (HB - 1) * P:HB * P], wtiles[a][:, H0:],
                    start=(a == 0), stop=(a == KT - 1),
                )
            osba = opool.tile([P, H0], f32, tag="osba", name="osba", bufs=1)
            nc.vector.tensor_tensor(osba[:], acc_a[:], brow[:, 0:H0],
                                    mybir.AluOpType.add)
            nc.scalar.dma_start(OUT.ap()[t * P:(t + 1) * P, 0:H0], osba[:])
            osbb = opool.tile([P, N - H0], f32, tag="osbb", name="osbb", bufs=1)
            nc.vector.tensor_tensor(osbb[:], acc_b[:], brow[:, H0:],
                                    mybir.AluOpType.add)
            nc.sync.dma_start(OUT.ap()[t * P:(t + 1) * P, H0:], osbb[:])

    nc.compile()
    return nc


def kernel(x, weight, bias, indx_seqs):
    x = np.asarray(x, dtype=np.float32)
    weight = np.asarray(weight, dtype=np.float32)
    bias = np.asarray(bias, dtype=np.float32)
    indx_seqs = np.asarray(indx_seqs)

    if "nc" not in _cache:
        _cache["nc"] = _build()
    nc = _cache["nc"]

    # Densify sparse weights: W'[o, i] += weight[o, k] at i = indx_seqs[o, k]
    wd = np.zeros((OUT_F, IN_F), dtype=np.float32)
    np.add.at(wd, (np.arange(OUT_F)[:, None], indx_seqs), weight)

    # Host pre-tiling into SBUF layouts, cast to bf16 (PE runs bf16 at the
    # same 1 col/cycle as fp32r; halves DMA; rel_err 3.0e-3 vs 2e-2 gate).
    # XK[h, p, (a*8+t)*128+c] = x[(h*8+t)*128+c, a*128+p]
    xk = np.ascontiguousarray(
        x.reshape(2, HB, P, KT, P).transpose(0, 4, 3, 1, 2)
    ).reshape(2, P, KT * HB * P).astype(ml_dtypes.bfloat16)
    in_maps = []
    for c in range(NCORES):
        wshard = wd[c * OSH:(c + 1) * OSH]            # (512, 4096)
        # WT[g, p, j*512+n] = W'[o0+n, (4g+j)*128+p]
        wt = np.ascontiguousarray(
            wshard.reshape(OSH, NG, WG, P).transpose(1, 3, 2, 0)
        ).reshape(NG, P, WG * OSH).astype(ml_dtypes.bfloat16)
        in_maps.append({
            "XK": xk,
            "WT": wt,
            "BIAS": np.ascontiguousarray(
                np.broadcast_to(bias[c * OSH:(c + 1) * OSH], (P, N))),
        })
    trace = bool(int(os.environ.get("BASSK_TRACE", "0"))) or bool(
        os.environ.get("BASS_TRACE"))
    if trace:
        _enable_ntff_hook()
    res = run_bass_kernel_spmd(
        nc, in_maps, list(range(NCORES)), trace=trace,
        trace_cores=list(range(NCORES)) if trace else None,
    )
    _cache["last_results"] = res

    out = np.concatenate([res.results[c]["OUT"] for c in range(NCORES)], axis=1)
    return out


# revision 9
# speedup vs baseline: 1.0164x; 1.0164x over previous
"""Trainium2 Bass kernel for nn_LinearCondensed.

Computes out[b, o] = sum_k weight[o, k] * x[b, indx_seqs[o, k]] + bias[o]
with B=2048, IN_F=OUT_F=4096, FAN_IN=32.

Strategy: densify the sparse weight matrix on the host --
W'[o, i] = sum_{k: indx_seqs[o,k]==i} weight[o, k] -- and run a dense bf16
matmul out = x @ W'^T + bias on the PE array. OUT_F is sharded 8 ways
across cores (512 columns each), x replicated. The kernel is PE-bound:
512 N=512-equivalent matmuls ~= 110.7us of back-to-back PE streaming at
the 512cyc/2.4GHz+NX roofline (measured 216ns/MM steady, LDWEIGHTS
hidden by the PE background weight buffer).

v3 (trace-driven): the head is DMA-gated, so x is host-tiled K-MAJOR in
two 8-b-tile interleaved halves. Phase 1 runs k-outer across 8 b-tiles:
each k-step needs one 256KB x k-slab + one 131KB weight chunk, so real
matmuls start as soon as ~390KB lands (~9us) instead of waiting 2.1MB
for the old 2-wide k-outer phase (~15.4us). Per-k-step feed is 224GB/s
vs ~280-358GB/s single-queue DMA, leaving slack everywhere after the
first step. Phase 2 = k-outer over b-tiles 8-14 (x half B streams in
far ahead); phase 3 = b-tile 15 k-inner in [384|128]-wide PSUM groups,
drained onto two different DMA queues so the post-matmul tail is just
two small stores + receipt. All 8 PSUM banks carry one "acc"-tagged
rotation (warm-up dummies share the rotation, so no dedicated bank).
Bias is folded into the PSUM drain. fp8 DoubleRow would be ~1.5x PE but
fails the 2e-2 gate (3-5e-2); bf16 measures rel_err 3.0e-3.
"""

import os
import sys
import types

import ml_dtypes
import numpy as np

import concourse.bacc as bacc
import concourse.mybir as mybir
import concourse.tile as tile
from concourse.bass_utils import run_bass_kernel_spmd

B, IN_F, OUT_F, FAN_IN = 2048, 4096, 4096, 32
NCORES = 8
OSH = OUT_F // NCORES          # 512 output features per core
P = 128                        # partitions
BT = B // P                    # 16 batch tiles
KT = IN_F // P                 # 32 contraction tiles
N = OSH                        # 512 moving columns (max for fp32 PSUM bank)
WG = 4                         # k-tiles per weight DMA group
NG = KT // WG                  # 8 weight groups
HB = BT // 2                   # 8 b-tiles per k-major half
NDUMMY = 24                    # N=256 warm-up matmuls bridge boot -> data (~5.1us)

f32 = mybir.dt.float32
bf16 = mybir.dt.bfloat16

_cache = {}


def _enable_ntff_hook():
    """Register the ctypes NTFF profile hook (the image's antenv lacks
    axon_hooks); lets trace=True produce a neuron-profile under axon."""
    try:
        from antenv.axon_hooks import get_axon_ntff_profile_hook  # noqa: F401
        return
    except ImportError:
        pass
    try:
        import antenv
        from trn_agent_boot.trn_boot import _ntff_profile_via_ctypes

        mod = types.ModuleType("antenv.axon_hooks")
        holder = [None]
        mod.set_axon_ntff_profile_hook = lambda h: holder.__setitem__(0, h)
        mod.get_axon_ntff_profile_hook = lambda: holder[0]
        antenv.axon_hooks = mod
        sys.modules["antenv.axon_hooks"] = mod
        mod.set_axon_ntff_profile_hook(
            _ntff_profile_via_ctypes("/opt/axon/libaxon_pjrt.so"))
        import concourse.bass_utils as bu
        bu.upload_artifacts = lambda tmpdir: str(tmpdir)
    except Exception:
        pass


def _build():
    nc = bacc.Bacc()
    # Layouts (host-pretiled, all contiguous per partition):
    #   XK[h, p, ((a*8 + t)*128 + c)] = x[(h*8 + t)*128 + c, a*128 + p]
    #     -> k-slab (h, a): [128, 8*128] = 256KB, one DMA
    #   WT[g, p, j*512 + n] = W'[o0 + n, (4g+j)*128 + p] -> group: [128, 2048]
    XK = nc.declare_dram_parameter("XK", [2, P, KT * HB * P], bf16, isOutput=False)
    WT = nc.declare_dram_parameter("WT", [NG, P, WG * N], bf16, isOutput=False)
    BIAS = nc.declare_dram_parameter("BIAS", [P, N], f32, isOutput=False)
    OUT = nc.declare_dram_parameter("OUT", [B, N], f32, isOutput=True)

    XKv = XK.ap().rearrange("h p (a r) -> h p a r", a=KT)   # r = t*128 + c

    with tile.TileContext(nc) as tc:
        with (
            tc.tile_pool(name="xpool", bufs=1) as xpool,
            tc.tile_pool(name="wpool", bufs=1) as wpool,
            tc.tile_pool(name="cpool", bufs=1) as cpool,
            tc.tile_pool(name="opool", bufs=8) as opool,
            tc.tile_pool(name="psum", bufs=8, space="PSUM") as psum,
        ):
            # PE p-state warmup while the first k-slab + weight chunk land.
            # Dummies write into the same "acc" PSUM rotation (no extra bank).
            dl = cpool.tile([P, P], bf16)
            dr = cpool.tile([P, 256], bf16)
            nc.vector.memset(dl[:], 0)
            nc.vector.memset(dr[:], 0)
            dacc = psum.tile([P, 256], f32, name="dacc", tag="acc")
            for _ in range(NDUMMY):
                nc.tensor.matmul(dacc[:], dl[:], dr[:], start=True, stop=True)

            xs = [xpool.tile([P, KT, HB * P], bf16, tag=f"xh{h}", name=f"xh{h}")
                  for h in range(2)]
            wgroups = [wpool.tile([P, WG * N], bf16, tag=f"w{g}", name=f"w{g}")
                       for g in range(NG)]
            brow = cpool.tile([P, N], f32)

            # ---- Input DMA program: one sync (HWDGE) queue, strict order.
            # Weight chunk covering k-tile a is always dispatched before
            # x k-slab a; consumption step a needs exactly (slab a, w[a]).
            def slab(h, a, eng=None):
                (eng or (nc.sync if a % 2 == 0 else nc.scalar)).dma_start(
                    xs[h][:, a, :], XKv[h][:, a, :])

            H4 = HB * P // 2
            nc.sync.dma_start(wgroups[0][:, 0:N], WT.ap()[0][:, 0:N])
            nc.scalar.dma_start(xs[0][:, 0, 0:H4], XKv[0][:, 0, 0:H4])
            nc.scalar.dma_start(xs[0][:, 0, H4:], XKv[0][:, 0, H4:])
            nc.sync.dma_start(wgroups[0][:, N:2 * N], WT.ap()[0][:, N:2 * N])
            nc.scalar.dma_start(xs[0][:, 1, 0:H4], XKv[0][:, 1, 0:H4])
            nc.scalar.dma_start(xs[0][:, 1, H4:], XKv[0][:, 1, H4:])
            nc.sync.dma_start(wgroups[0][:, 2 * N:], WT.ap()[0][:, 2 * N:])
            slab(0, 2, nc.sync)
            slab(0, 3, nc.scalar)
            for g in range(1, NG):
                nc.sync.dma_start(wgroups[g][:], WT.ap()[g])
                for a in range(WG * g, WG * (g + 1)):
                    slab(0, a)
            nc.sync.dma_start(brow[:], BIAS.ap())
            for a in range(KT):
                slab(1, a)

            wtiles = [wgroups[a // WG][:, (a % WG) * N:(a % WG + 1) * N]
                      for a in range(KT)]

            # bias folded into the PSUM drain (bias row pre-replicated
            # across partitions on host)
            def finish_tile(t, acc, queue=None):
                osb = opool.tile([P, N], f32, tag="osb", name="osb")
                nc.vector.tensor_tensor(osb[:], acc[:], brow[:], mybir.AluOpType.add)
                (queue or nc.scalar).dma_start(OUT.ap()[t * P:(t + 1) * P, :], osb[:])

            # Phase 1: k-outer across b-tiles 0-7 (8 PSUM banks), consuming
            # each (slab, w-chunk) pair as it lands. 224GB/s steady demand.
            accs1 = [psum.tile([P, N], f32, name=f"acc{t}", tag="acc")
                     for t in range(HB)]
            for a in range(KT):
                for t in range(HB):
                    nc.tensor.matmul(
                        accs1[t][:], xs[0][:, a, t * P:(t + 1) * P], wtiles[a][:],
                        start=(a == 0), stop=(a == KT - 1),
                    )
            for t in range(HB):
                finish_tile(t, accs1[t])

            # Phase 2: k-outer across b-tiles 8-14; half-B slabs landed long
            # ago (all input DMA completes ~65us, phase 2 ends ~113us).
            accs2 = [psum.tile([P, N], f32, name=f"acc{HB + t}", tag="acc")
                     for t in range(HB - 1)]
            for a in range(KT):
                for t in range(HB - 1):
                    nc.tensor.matmul(
                        accs2[t][:], xs[1][:, a, t * P:(t + 1) * P], wtiles[a][:],
                        start=(a == 0), stop=(a == KT - 1),
                    )
            for t in range(HB - 1):
                finish_tile(HB + t, accs2[t])

            # Phase 3: last b-tile k-inner in [384|128]-wide groups; drains
            # go to two different DMA queues so the tail after the very
            # last matmul is two small parallel stores.
            t = BT - 1
            H0 = 384
            acc_a = psum.tile([P, H0], f32, name="acca", tag="acc")
            acc_b = psum.tile([P, N - H0], f32, name="accb", tag="acc")
            for a in range(KT):
                nc.tensor.matmul(
                    acc_a[:], xs[1][:, a, (HB - 1) * P:HB * P], wtiles[a][:, 0:H0],
                    start=(a == 0), stop=(a == KT - 1),
                )
                nc.tensor.matmul(
                    acc_b[:], xs[1][:, a, (HB - 1) * P:HB * P], wtiles[a][:, H0:],
                    start=(a == 0), stop=(a == KT - 1),
                )
            osba = opool.tile([P, H0], f32, tag="osba", name="osba", bufs=1)
            nc.vector.tensor_tensor(osba[:], acc_a[:], brow[:, 0:H0],
                                    mybir.AluOpType.add)
            nc.scalar.dma_start(OUT.ap()[t * P:(t + 1) * P, 0:H0], osba[:])
            osbb = opool.tile([P, N - H0], f32, tag="osbb", name="osbb", bufs=1)
            nc.vector.tensor_tensor(osbb[:], acc_b[:], brow[:, H0:],
                                    mybir.AluOpType.add)
            nc.sync.dma_start(OUT.ap()[t * P:(t + 1) * P, H0:], osbb[:])

    nc.compile()
    return nc


def kernel(x, weight, bias, indx_seqs):
    x = np.asarray(x, dtype=np.float32)
    weight = np.asarray(weight, dtype=np.float32)
    bias = np.asarray(bias, dtype=np.float32)
    indx_seqs = np.asarray(indx_seqs)

    if "nc" not in _cache:
        _cache["nc"] = _build()
    nc = _cache["nc"]

    # Densify sparse weights: W'[o, i] += weight[o, k] at i = indx_seqs[o, k]
    wd = np.zeros((OUT_F, IN_F), dtype=np.float32)
    np.add.at(wd, (np.arange(OUT_F)[:, None], indx_seqs), weight)

    # Host pre-tiling into SBUF layouts, cast to bf16 (PE runs bf16 at the
    # same 1 col/cycle as fp32r; halves DMA; rel_err 3.0e-3 vs 2e-2 gate).
    # XK[h, p, (a*8+t)*128+c] = x[(h*8+t)*128+c, a*128+p]
    xk = np.ascontiguousarray(
        x.reshape(2, HB, P, KT, P).transpose(0, 4, 3, 1, 2)
    ).reshape(2, P, KT * HB * P).astype(ml_dtypes.bfloat16)
    in_maps = []
    for c in range(NCORES):
        wshard = wd[c * OSH:(c + 1) * OSH]            # (512, 4096)
        # WT[g, p, j*512+n] = W'[o0+n, (4g+j)*128+p]
        wt = np.ascontiguousarray(
            wshard.reshape(OSH, NG, WG, P).transpose(1, 3, 2, 0)
        ).reshape(NG, P, WG * OSH).astype(ml_dtypes.bfloat16)
        in_maps.append({
            "XK": xk,
            "WT": wt,
            "BIAS": np.ascontiguousarray(
                np.broadcast_to(bias[c * OSH:(c + 1) * OSH], (P, N))),
        })
    trace = bool(int(os.environ.get("BASSK_TRACE", "0"))) or bool(
        os.environ.get("BASS_TRACE"))
    if trace:
        _enable_ntff_hook()
    res = run_bass_kernel_spmd(
        nc, in_maps, list(range(NCORES)), trace=trace,
        trace_cores=list(range(NCORES)) if trace else None,
    )
    _cache["last_results"] = res

    out = np.concatenate([res.results[c]["OUT"] for c in range(NCORES)], axis=1)
    return out
